# revision 28
# baseline (speedup 1.0000x reference)
"""Multi-head attention (B=2, S=2048, D=1024, H=16) on 8 NeuronCores.

Sharding: core c -> batch b = c//4, head group g = c%4 (4 heads each).
Each core computes q/k/v projections for its head group, full softmax
attention for its 4 heads, and a partial output projection
out_c = attn_out_c @ Wo[rows_c].  The host sums the 4 partials per batch
and adds bo.

v2 design (from the v1 fp32r kernel's trace: PE busy 204us of 251us,
ScalarE exp chain 147us, 12us exposed tail epilogue, 20us DMA prologue):
  - all inputs bf16 (halves the 12.6MB input DMA -> prologue and
    mid-kernel HBM pressure), qT/kT/outTs/weights bf16 on SBUF.  PE
    streams bf16 at 1 col/cycle like fp32r, so matmul time is unchanged
    by dtype; correctness headroom (gate 2e-2) allows it.
  - attention probabilities and v in bf16 (fp8e4 DoubleRow was tried:
    -27us of PE matmul columns, but e4m3's 1.8% rms quantization on p
    and v passes straight to the output -- softmax averaging shrinks
    signal and noise equally -- giving 2.2e-2 rel err > the 2e-2 gate,
    and the fp8-dst ACTIVATE cost +220ns/instr on the exp chain).
  - denominator via 64 ones-COLUMNS in the stationary (v2[...,64:128]):
    psum rows 0:64 = unnormalized attn out, rows 64:128 = denominator
    replicated -- same matmul column count, but the epilogue becomes 3
    DVE ops (cross-quadrant den copy 64->0, reciprocal_approx_fast,
    normalize-mul written straight into pair-packed outTs at base 0/64).
    Replaces v1's copy+DMA-shift+gpsimd-broadcast+recip+mul+2 DMA chain
    that exposed ~12us at the kernel tail.
  - schedule: group (hp0,sqc0) interleaved into the QKV phase, the
    other 7 groups stream at the exp cadence (ScalarE exp = 128 x
    1.1us ACTIVATEs ~= 142us busy; PE ~= 176us busy -- the two chains
    are nearly balanced, so filler placement matters: later groups are
    ACT-bound by ~4us each, and carry the q projections for sqc2/3 and
    all the output-projection pieces as fillers; k projections ride in
    phase 1 and group (1,0)).
  - output projection: 2-matmul pieces; both halves of an output row
    block evacuate into one [128,1024] tile for a single paired DMA;
    out is bf16 (host upcasts and sums partials).  For the last sqc the
    hp0-half matmuls run inside group (1,3) with results parked in SBUF
    (no PSUM slot held), and the hp1 halves + DVE add form the tail,
    each half's out DMA firing unpaired as soon as its add lands --
    ~10us from last exp to done.
  - ~7 dummy warmup matmuls during the DMA prologue trip the HAM
    activity monitor so the first real chains run at 2.4GHz.
  - PSUM (8 banks): 2x scores [128,2,512] (4) + 1x attn accumulator pair
    [128,512]x2 (2) + 2 shared qkv/proj accumulators (2).

  Measured: ~210us HW exec (v1 fp32r baseline: ~249us); rel err 5.3e-3.
  Known residual: the PE HAM clock re-throttles to 1.2GHz for ~3.4us
  once per ACT-bound group (boundary DVE backlog starves proj fillers);
  LDWEIGHTS-dummy keep-alives made it worse (they serialize against
  in-flight matmuls), fp8 attn@v fails the accuracy gate -- see above.
"""

import numpy as np
import ml_dtypes

S = 2048
D = 1024
H = 16
DEPTH = 64
NCORES = 8
GH = 4              # heads per core
GD = GH * DEPTH     # 256 output dims per core
KC = 8              # contraction chunks of 128 (1024 = D)
BF16 = ml_dtypes.bfloat16

_state = {}


def _build():
    import concourse.mybir as mybir
    import concourse.tile as tile
    from concourse import bacc
    from concourse.bass import ts

    fp32 = mybir.dt.float32
    bf16 = mybir.dt.bfloat16
    fp8 = mybir.dt.float8e4
    Exp = mybir.ActivationFunctionType.Exp
    Add = mybir.AluOpType.add
    Mult = mybir.AluOpType.mult
    DR = mybir.MatmulPerfMode.DoubleRow

    nc = bacc.Bacc("TRN2", target_bir_lowering=False, debug=False)
    # all inputs pre-packed on the host to the SBUF layout (partition first)
    xp = nc.dram_tensor("xp", [128, 4, KC, 512], bf16, kind="ExternalInput")
    wq = nc.dram_tensor("wq", [128, KC, GD], bf16, kind="ExternalInput")
    wk = nc.dram_tensor("wk", [128, KC, GD], bf16, kind="ExternalInput")
    wv = nc.dram_tensor("wv", [128, KC, GD], bf16, kind="ExternalInput")
    wo = nc.dram_tensor("wo", [128, 2, D], bf16, kind="ExternalInput")
    # bias[:, 0:256] = bv broadcast, [:, 256:258] = bq by (p, hp), 258:260 bk
    bias = nc.dram_tensor("bias", [128, 260], fp32, kind="ExternalInput")
    out = nc.dram_tensor("out", [S, D], bf16, kind="ExternalOutput")

    with tile.TileContext(nc) as tc:
        with (
            tc.tile_pool(name="singles", bufs=1) as singles,
            tc.tile_pool(name="xpool", bufs=4) as xpool,
            tc.tile_pool(name="expp", bufs=4) as expp,
            tc.tile_pool(name="rqp", bufs=2) as rqp,
            tc.tile_pool(name="otp", bufs=4) as otp,
            tc.tile_pool(name="htp", bufs=8) as htp,
        ):
            qT = singles.tile([128, 2, S], bf16)        # [dout%128, pair, sq]
            kT = singles.tile([128, 2, S], bf16)
            # v2[p, kb, h, 0:64] = v_h[key=kb*128+p, :]; cols 64:128 = 1
            v2 = singles.tile([128, 16, GH, 128], bf16)
            outTs = singles.tile([128, 2, S], bf16)     # pair-packed attn out
            wq_sb = singles.tile([128, KC, GD], bf16)
            wk_sb = singles.tile([128, KC, GD], bf16)
            wv_sb = singles.tile([128, KC, GD], bf16)
            wo_sb = singles.tile([128, 2, D], bf16)     # pair-packed Wo rows
            bias_sb = singles.tile([128, 260], fp32)
            scr = singles.tile([1, 16], fp32)

            nc.vector.memset(v2[:, :, :, 64:128], 1.0)
            warm = singles.tile([128, 512], bf16)
            nc.vector.memset(warm[:], 0.0)
            # prime the ScalarE exp table load during the DMA prologue
            nc.vector.memset(scr[:], 0.0)
            nc.scalar.activation(scr[0:1, 8:16], scr[0:1, 0:8], Exp)

            # ---------------- input DMAs (priority ~ emission order) -------
            # wq then x(sc0) first: the first q matmul group needs exactly
            # those two, so compute starts early while wk/wv stream behind.
            nc.sync.dma_start(wk_sb[:, :, 0:128], wk[:, :, 0:128])
            xc0 = xpool.tile([128, KC, 512], bf16, tag="xc")
            nc.sync.dma_start(xc0[:, 0:4, :], xp[:, 0, 0:4, :])
            nc.sync.dma_start(wq_sb[:, :, 0:128], wq[:, :, 0:128])
            nc.sync.dma_start(xc0[:, 4:KC, :], xp[:, 0, 4:KC, :])
            nc.sync.dma_start(bias_sb[:], bias[:])

            # PE warmup: ~7 dummy matmuls run while the prologue DMAs land,
            # tripping the HAM activity monitor so the first real chains
            # start at 2.4GHz instead of 1.2GHz.
            with tc.tile_pool(name="pwarm", bufs=1, space="PSUM") as pwarm:
                wps = pwarm.tile([128, 512], fp32)
                for _ in range(7):
                    nc.tensor.matmul(
                        wps[:], warm[:, 0:128], warm[:], start=True, stop=True
                    )
                nc.vector.tensor_copy(warm[:], wps[:])
            nc.sync.dma_start(wk_sb[:, :, 128:GD], wk[:, :, 128:GD])
            nc.sync.dma_start(wq_sb[:, :, 128:GD], wq[:, :, 128:GD])
            nc.sync.dma_start(wv_sb[:], wv[:])

            # ---------------- emission helpers ----------------------------
            def emit_xc(sc):
                xc = xpool.tile([128, KC, 512], bf16, tag="xc")
                nc.sync.dma_start(xc[:, 0:4, :], xp[:, sc, 0:4, :])
                nc.sync.dma_start(xc[:, 4:KC, :], xp[:, sc, 4:KC, :])
                return xc

            def emit_qk(sc, hp, which, xc, paux):
                """One q-or-k projection group for (s-chunk, head pair)."""
                w_sb, dstT, bcol = (
                    (wq_sb, qT, 256) if which == "q" else (wk_sb, kT, 258)
                )
                ps = paux.tile([128, 512], fp32, tag="aux")
                for kc in range(KC):
                    nc.tensor.matmul(
                        ps[:],
                        w_sb[:, kc, ts(hp, 128)],
                        xc[:, kc, :],
                        start=(kc == 0),
                        stop=(kc == KC - 1),
                    )
                nc.vector.tensor_tensor(
                    dstT[:, hp, ts(sc, 512)],
                    ps[:],
                    bias_sb[:, bcol + hp : bcol + hp + 1].to_broadcast(
                        [128, 512]
                    ),
                    Add,
                )

            def qk_thunks(sc, hp, which, xc, paux):
                """A q/k projection as two 4-matmul half-chain thunks (the
                PSUM accumulation spans the two emission points), so filler
                insertion never blocks the attention stream for more than
                ~1us."""
                w_sb, dstT, bcol = (
                    (wq_sb, qT, 256) if which == "q" else (wk_sb, kT, 258)
                )
                box = {}
                def t1():
                    ps = paux.tile([128, 512], fp32, tag="aux")
                    box["ps"] = ps
                    for kc in range(4):
                        nc.tensor.matmul(
                            ps[:],
                            w_sb[:, kc, ts(hp, 128)],
                            xc[:, kc, :],
                            start=(kc == 0),
                            stop=False,
                        )
                def t2():
                    ps = box["ps"]
                    for kc in range(4, KC):
                        nc.tensor.matmul(
                            ps[:],
                            w_sb[:, kc, ts(hp, 128)],
                            xc[:, kc, :],
                            start=False,
                            stop=(kc == KC - 1),
                        )
                    nc.vector.tensor_tensor(
                        dstT[:, hp, ts(sc, 512)],
                        ps[:],
                        bias_sb[:, bcol + hp : bcol + hp + 1].to_broadcast(
                            [128, 512]
                        ),
                        Add,
                    )
                return [t1, t2]

            def emit_v(sc, xc, paux):
                for mm in range(4):
                    kb = sc * 4 + mm
                    ps = paux.tile([128, GD], fp32, tag="aux")
                    for kc in range(KC):
                        nc.tensor.matmul(
                            ps[:],
                            xc[:, kc, ts(mm, 128)],
                            wv_sb[:, kc, :],
                            start=(kc == 0),
                            stop=(kc == KC - 1),
                        )
                    nc.vector.tensor_tensor(
                        v2[:, kb, :, 0:DEPTH],
                        ps[:].rearrange("p (h d) -> p h d", h=GH),
                        bias_sb[:, 0:256].rearrange("p (h d) -> p h d", h=GH),
                        Add,
                    )

            def emit_sexp(hp, sqc, kb, pss):
                """Scores pair -> exp for one k-block (needs only qT/kT)."""
                sps = pss.tile([128, 2, 512], fp32, tag="s")
                for a in range(2):
                    nc.tensor.matmul(
                        sps[:, a, :],
                        kT[a * 64 : (a + 1) * 64, hp, ts(kb, 128)],
                        qT[a * 64 : (a + 1) * 64, hp, ts(sqc, 512)],
                        start=True,
                        stop=True,
                    )
                ex = expp.tile([128, 2, 512], bf16, tag="e")
                nc.scalar.activation(ex[:], sps[:], Exp, scale=0.125)
                return ex

            def emit_attnv(hp, kb, oab, ex):
                for a in range(2):
                    nc.tensor.matmul(
                        oab[a][:],
                        v2[:, kb, 2 * hp + a, :],
                        ex[:, a, :],
                        start=(kb == 0),
                        stop=(kb == 15),
                    )

            def endgroup(hp, sqc, oab):
                """Normalize one finished group (off critical path).

                oab rows 0:64 = unnormalized attention out, rows 64:128 =
                softmax denominator (64 ones-columns in v2).  Cross-quadrant
                DVE copy moves the denominator block to partitions 0:64
                (compute ops cannot read cross-quadrant, plain copies can),
                reciprocal_approx_fast (~51 ULP, plenty for softmax), then
                the normalize-mul writes straight into pair-packed outTs
                (out base 0 for head a=0, base 64 for a=1 -- DVE writes to
                either half-quadrant from base-0 inputs).
                """
                for a in range(2):
                    den = rqp.tile([64, 512], fp32, tag="rq")
                    rcp = rqp.tile([64, 512], fp32, tag="rc")
                    nc.vector.tensor_copy(den[:], oab[a][64:128, :])
                    nc.vector.reciprocal_approx_fast(rcp[:], den[:])
                    nc.vector.tensor_tensor(
                        outTs[a * 64 : (a + 1) * 64, hp, ts(sqc, 512)],
                        oab[a][0:64, :],
                        rcp[:],
                        Mult,
                    )

            def proj_pieces(sqc, paux, split=True):
                """Output projection for one sqc as a list of emission thunks.

                With split=True each (m, nn) piece becomes two thunks (one
                matmul each) so interleaved filler work never blocks the
                attention scores for more than ~one matmul.
                """
                pieces = []
                ot2box = {}
                for mm in range(4):
                    m = sqc * 4 + mm
                    for nn in range(2):
                        box = {}
                        def start(m=m, nn=nn, box=box):
                            ps = paux.tile([128, 512], fp32, tag="aux")
                            box["ps"] = ps
                            nc.tensor.matmul(
                                ps[:],
                                outTs[:, 0, ts(m, 128)],
                                wo_sb[:, 0, ts(nn, 512)],
                                start=True,
                                stop=False,
                            )
                        def finish(m=m, nn=nn, box=box, ot2box=ot2box):
                            ps = box["ps"]
                            nc.tensor.matmul(
                                ps[:],
                                outTs[:, 1, ts(m, 128)],
                                wo_sb[:, 1, ts(nn, 512)],
                                start=False,
                                stop=True,
                            )
                            if nn == 0:
                                ot2 = otp.tile([128, 1024], bf16, tag="ot",
                                               name="ot2")
                                ot2box[m] = ot2
                            ot2 = ot2box[m]
                            nc.vector.tensor_copy(ot2[:, ts(nn, 512)], ps[:])
                            if nn == 1:
                                nc.sync.dma_start(
                                    out[m * 128 : (m + 1) * 128, :], ot2[:]
                                )
                        if split:
                            pieces.append(start)
                            pieces.append(finish)
                        else:
                            def whole(s=start, f=finish):
                                s(); f()
                            pieces.append(whole)
                return pieces

            def proj_tail_pieces(paux):
                """sqc3 projection split for the kernel tail: the hp0-half
                matmuls run as fillers inside group (1,3) (start/stop=True,
                result parked in SBUF so no PSUM slot stays held); the
                hp1 halves + DVE add + paired out DMA run after group
                (1,3)'s epilogue."""
                sqc = 3
                starts, finishes = [], []
                ot2box = {}
                for mm in range(4):
                    m = sqc * 4 + mm
                    for nn in range(2):
                        hbox = {}
                        def start(m=m, nn=nn, hbox=hbox):
                            ps = paux.tile([128, 512], fp32, tag="aux")
                            nc.tensor.matmul(
                                ps[:],
                                outTs[:, 0, ts(m, 128)],
                                wo_sb[:, 0, ts(nn, 512)],
                                start=True,
                                stop=True,
                            )
                            ht = htp.tile([128, 512], fp32, tag="ht")
                            nc.vector.tensor_copy(ht[:], ps[:])
                            hbox["ht"] = ht
                        def finish(m=m, nn=nn, hbox=hbox):
                            ps = paux.tile([128, 512], fp32, tag="aux")
                            nc.tensor.matmul(
                                ps[:],
                                outTs[:, 1, ts(m, 128)],
                                wo_sb[:, 1, ts(nn, 512)],
                                start=True,
                                stop=True,
                            )
                            ot = otp.tile([128, 1024], bf16, tag="ot",
                                          name="ot")
                            nc.vector.tensor_tensor(
                                ot[:, 0:512], ps[:], hbox["ht"][:], Add
                            )
                            # unpaired DMA: in the tail, latency beats
                            # descriptor-count -- fire as soon as the add
                            # lands instead of waiting for the m-block pair
                            nc.sync.dma_start(
                                out[m * 128 : (m + 1) * 128, ts(nn, 512)],
                                ot[:, 0:512],
                            )
                        starts.append(start)
                        finishes.append(finish)
                return starts, finishes

            def emit_group(hp, sqc, pss, poab, fillers=()):
                """16 k-blocks + epilogue; one filler per k-block."""
                oab = [
                    poab.tile([128, 512], fp32, tag="o", name=f"o{a}")
                    for a in range(2)
                ]
                fill = list(fillers)
                fi = 0
                for kb in range(16):
                    ex = emit_sexp(hp, sqc, kb, pss)
                    emit_attnv(hp, kb, oab, ex)
                    if fi < len(fill):
                        fill[fi]()
                        fi += 1
                while fi < len(fill):
                    fill[fi]()
                    fi += 1
                endgroup(hp, sqc, oab)

            # ---------------- main emission --------------------------------
            # group (hp0, sqc0) is interleaved into phase 1: its k-block j
            # only needs kT/v for s-chunk j//4.  Units are emitted right
            # after the chunk's q/k projections (before the v projections)
            # so the first exp fires as early as possible; the attn@v
            # matmuls inside each unit wait on the v evacuations via
            # dataflow.
            with (
                tc.tile_pool(name="pss", bufs=2, space="PSUM") as pss,
                tc.tile_pool(name="poab", bufs=2, space="PSUM") as poab,
                tc.tile_pool(name="paux", bufs=2, space="PSUM") as paux,
            ):
                oab00 = [
                    poab.tile([128, 512], fp32, tag="o", name=f"o{a}")
                    for a in range(2)
                ]
                xcs = [xc0] + [emit_xc(sc) for sc in range(1, 4)]
                nc.sync.dma_start(wo_sb[:], wo[:])
                for sc in range(4):
                    xc = xcs[sc]
                    emit_qk(sc, 0, "k", xc, paux)
                    if sc == 0:
                        emit_qk(0, 0, "q", xc, paux)
                    exs = [
                        emit_sexp(0, 0, kb, pss)
                        for kb in range(4 * sc, 4 * sc + 4)
                    ]
                    if sc == 0:
                        emit_qk(0, 1, "k", xc, paux)
                        emit_qk(0, 1, "q", xc, paux)
                    if sc == 2:
                        emit_qk(1, 0, "q", xcs[1], paux)
                    if sc == 3:
                        emit_qk(1, 1, "q", xcs[1], paux)
                    emit_v(sc, xc, paux)
                    for kb, ex in zip(range(4 * sc, 4 * sc + 4), exs):
                        emit_attnv(0, kb, oab00, ex)
                endgroup(0, 0, oab00)

                # group (hp1, sqc0): k(hp1) for s-chunks 1-3 produced just
                # ahead of the k-blocks that consume them.  Later groups are
                # ACT-bound (group matmuls 13.6us < 17.7us of exp), so the
                # filler load is spread to match: q projections fill the
                # sqc1 groups, proj pieces fill the sqc2/3 groups.
                fill10 = (
                    qk_thunks(1, 1, "k", xcs[1], paux)
                    + qk_thunks(2, 1, "k", xcs[2], paux)
                    + qk_thunks(3, 1, "k", xcs[3], paux)
                )
                emit_group(1, 0, pss, poab, fill10)

                emit_group(
                    0, 1, pss, poab,
                    qk_thunks(2, 0, "q", xcs[2], paux)
                    + qk_thunks(2, 1, "q", xcs[2], paux),
                )
                emit_group(
                    1, 1, pss, poab,
                    qk_thunks(3, 0, "q", xcs[3], paux)
                    + qk_thunks(3, 1, "q", xcs[3], paux),
                )
                pp0 = proj_pieces(0, paux)
                emit_group(0, 2, pss, poab, pp0)
                pp1 = proj_pieces(1, paux)
                emit_group(1, 2, pss, poab, pp1)
                pp2 = proj_pieces(2, paux)
                emit_group(0, 3, pss, poab, pp2)
                pp3s, pp3f = proj_tail_pieces(paux)
                emit_group(1, 3, pss, poab, pp3s)
                for t in pp3f:
                    t()

    nc.compile()
    return nc


def _get_nc():
    if "nc" not in _state:
        _state["nc"] = _build()
    return _state["nc"]


def _prep_core_inputs(inputs, Wq, bq, Wk, bk, Wv, bv, Wo, bo):
    """Host-side shard + pack into the exact SBUF layouts (contiguous DMAs)."""
    in_maps = []
    xps = [
        np.ascontiguousarray(
            # [S, D] -> [p, sc, c, s']: x[sc*512+s', c*128+p]
            np.asarray(inputs[b], np.float32)
            .reshape(4, 512, KC, 128)
            .transpose(3, 0, 2, 1)
            .astype(BF16)
        )
        for b in range(2)
    ]
    for c in range(NCORES):
        b, g = divmod(c, 4)
        cols = slice(g * GD, (g + 1) * GD)
        bias_pack = np.empty((128, 260), np.float32)
        bias_pack[:, 0:256] = np.asarray(bv, np.float32)[None, cols]
        bias_pack[:, 256:258] = np.asarray(bq, np.float32)[cols].reshape(2, 128).T
        bias_pack[:, 258:260] = np.asarray(bk, np.float32)[cols].reshape(2, 128).T
        m = {
            "xp": xps[b],
            # [D, GD] -> [p, c, d]: W[c*128+p, d]
            "wq": np.ascontiguousarray(
                np.asarray(Wq, np.float32)[:, cols].reshape(KC, 128, GD).transpose(1, 0, 2).astype(BF16)
            ),
            "wk": np.ascontiguousarray(
                np.asarray(Wk, np.float32)[:, cols].reshape(KC, 128, GD).transpose(1, 0, 2).astype(BF16)
            ),
            "wv": np.ascontiguousarray(
                np.asarray(Wv, np.float32)[:, cols].reshape(KC, 128, GD).transpose(1, 0, 2).astype(BF16)
            ),
            # [GD, D] -> [p, hp, n]: Wo[hp*128+p, n]
            "wo": np.ascontiguousarray(
                np.asarray(Wo, np.float32)[cols, :].reshape(2, 128, D).transpose(1, 0, 2).astype(BF16)
            ),
            "bias": bias_pack,
        }
        in_maps.append(m)
    return in_maps


def run(inputs, Wq, bq, Wk, bk, Wv, bv, Wo, bo, trace=False):
    from concourse.bass_utils import run_bass_kernel_spmd

    nc = _get_nc()
    in_maps = _prep_core_inputs(inputs, Wq, bq, Wk, bk, Wv, bv, Wo, bo)
    res = run_bass_kernel_spmd(
        nc, in_maps, core_ids=list(range(NCORES)), trace=trace
    )
    out = np.zeros((2, S, D), np.float32)
    for c in range(NCORES):
        out[c // 4] += res.results[c]["out"]
    out += np.asarray(bo, np.float32)
    return out, res


def kernel(inputs, Wq, bq, Wk, bk, Wv, bv, Wo, bo):
    out, _ = run(
        np.asarray(inputs, np.float32),
        np.asarray(Wq, np.float32), np.asarray(bq, np.float32),
        np.asarray(Wk, np.float32), np.asarray(bk, np.float32),
        np.asarray(Wv, np.float32), np.asarray(bv, np.float32),
        np.asarray(Wo, np.float32), np.asarray(bo, np.float32),
    )
    return out


# revision 31
# speedup vs baseline: 1.0206x; 1.0206x over previous
"""Multi-head attention (B=2, S=2048, D=1024, H=16) on 8 NeuronCores.

Sharding: core c -> batch b = c//4, head group g = c%4 (4 heads each).
Each core computes q/k/v projections for its head group, full softmax
attention for its 4 heads, and a partial output projection
out_c = attn_out_c @ Wo[rows_c].  The host sums the 4 partials per batch
and adds bo.

v2 design (from the v1 fp32r kernel's trace: PE busy 204us of 251us,
ScalarE exp chain 147us, 12us exposed tail epilogue, 20us DMA prologue):
  - all inputs bf16 (halves the 12.6MB input DMA -> prologue and
    mid-kernel HBM pressure), qT/kT/outTs/weights bf16 on SBUF.  PE
    streams bf16 at 1 col/cycle like fp32r, so matmul time is unchanged
    by dtype; correctness headroom (gate 2e-2) allows it.
  - attention probabilities and v in bf16 (fp8e4 DoubleRow was tried:
    -27us of PE matmul columns, but e4m3's 1.8% rms quantization on p
    and v passes straight to the output -- softmax averaging shrinks
    signal and noise equally -- giving 2.2e-2 rel err > the 2e-2 gate,
    and the fp8-dst ACTIVATE cost +220ns/instr on the exp chain).
  - denominator via 64 ones-COLUMNS in the stationary (v2[...,64:128]):
    psum rows 0:64 = unnormalized attn out, rows 64:128 = denominator
    replicated -- same matmul column count, but the epilogue becomes 3
    DVE ops (cross-quadrant den copy 64->0, reciprocal_approx_fast,
    normalize-mul written straight into pair-packed outTs at base 0/64).
    Replaces v1's copy+DMA-shift+gpsimd-broadcast+recip+mul+2 DMA chain
    that exposed ~12us at the kernel tail.
  - schedule: group (hp0,sqc0) interleaved into the QKV phase, the
    other 7 groups stream at the exp cadence (ScalarE exp = 128 x
    1.1us ACTIVATEs ~= 142us busy; PE ~= 176us busy -- the two chains
    are nearly balanced, so filler placement matters: later groups are
    ACT-bound by ~4us each, and carry the q projections for sqc2/3 and
    all the output-projection pieces as fillers; k projections ride in
    phase 1 and group (1,0)).
  - output projection: 2-matmul pieces; both halves of an output row
    block evacuate into one [128,1024] tile for a single paired DMA;
    out is bf16 (host upcasts and sums partials).  For the last sqc the
    hp0-half matmuls run inside group (1,3) with results parked in SBUF
    (no PSUM slot held), and the hp1 halves + DVE add form the tail,
    each half's out DMA firing unpaired as soon as its add lands --
    ~10us from last exp to done.
  - 12 dummy warmup matmuls during the DMA prologue trip the HAM
    activity monitor so the first real chains run at 2.4GHz; the warm
    tile's memset is emitted before the big v2-ones memset so the
    warmup isn't queued behind 4us of DVE work.
  - PSUM (8 banks): 2x scores [128,2,512] (4) + 1x attn accumulator pair
    [128,512]x2 (2) + 2 shared qkv/proj accumulators (2).

  Measured: ~210us HW exec (v1 fp32r baseline: ~249us); rel err 5.3e-3.
  Known residual: the PE HAM clock re-throttles to 1.2GHz for ~3.4us
  once per ACT-bound group (boundary DVE backlog starves proj fillers);
  LDWEIGHTS-dummy keep-alives made it worse (they serialize against
  in-flight matmuls), fp8 attn@v fails the accuracy gate -- see above.
"""

import numpy as np
import ml_dtypes

S = 2048
D = 1024
H = 16
DEPTH = 64
NCORES = 8
GH = 4              # heads per core
GD = GH * DEPTH     # 256 output dims per core
KC = 8              # contraction chunks of 128 (1024 = D)
BF16 = ml_dtypes.bfloat16

_state = {}


def _build():
    import concourse.mybir as mybir
    import concourse.tile as tile
    from concourse import bacc
    from concourse.bass import ts

    fp32 = mybir.dt.float32
    bf16 = mybir.dt.bfloat16
    fp8 = mybir.dt.float8e4
    Exp = mybir.ActivationFunctionType.Exp
    Add = mybir.AluOpType.add
    Mult = mybir.AluOpType.mult
    DR = mybir.MatmulPerfMode.DoubleRow

    nc = bacc.Bacc("TRN2", target_bir_lowering=False, debug=False)
    # all inputs pre-packed on the host to the SBUF layout (partition first)
    xp = nc.dram_tensor("xp", [128, 4, KC, 512], bf16, kind="ExternalInput")
    wq = nc.dram_tensor("wq", [128, KC, GD], bf16, kind="ExternalInput")
    wk = nc.dram_tensor("wk", [128, KC, GD], bf16, kind="ExternalInput")
    wv = nc.dram_tensor("wv", [128, KC, GD], bf16, kind="ExternalInput")
    wo = nc.dram_tensor("wo", [128, 2, D], bf16, kind="ExternalInput")
    # bias[:, 0:256] = bv broadcast, [:, 256:258] = bq by (p, hp), 258:260 bk
    bias = nc.dram_tensor("bias", [128, 260], fp32, kind="ExternalInput")
    out = nc.dram_tensor("out", [S, D], bf16, kind="ExternalOutput")

    with tile.TileContext(nc) as tc:
        with (
            tc.tile_pool(name="singles", bufs=1) as singles,
            tc.tile_pool(name="xpool", bufs=4) as xpool,
            tc.tile_pool(name="expp", bufs=4) as expp,
            tc.tile_pool(name="rqp", bufs=2) as rqp,
            tc.tile_pool(name="otp", bufs=4) as otp,
            tc.tile_pool(name="htp", bufs=8) as htp,
        ):
            qT = singles.tile([128, 2, S], bf16)        # [dout%128, pair, sq]
            kT = singles.tile([128, 2, S], bf16)
            # v2[p, kb, h, 0:64] = v_h[key=kb*128+p, :]; cols 64:128 = 1
            v2 = singles.tile([128, 16, GH, 128], bf16)
            outTs = singles.tile([128, 2, S], bf16)     # pair-packed attn out
            wq_sb = singles.tile([128, KC, GD], bf16)
            wk_sb = singles.tile([128, KC, GD], bf16)
            wv_sb = singles.tile([128, KC, GD], bf16)
            wo_sb = singles.tile([128, 2, D], bf16)     # pair-packed Wo rows
            bias_sb = singles.tile([128, 260], fp32)
            scr = singles.tile([1, 16], fp32)

            # warm-tile memset FIRST: the v2 ones memset is ~4.3us of DVE
            # and would otherwise delay the PE warmup matmuls past the
            # point where the first real chains' DMAs land.
            warm = singles.tile([128, 512], bf16)
            nc.vector.memset(warm[:], 0.0)
            # prime the ScalarE exp table load during the DMA prologue
            nc.vector.memset(scr[:], 0.0)
            nc.scalar.activation(scr[0:1, 8:16], scr[0:1, 0:8], Exp)
            nc.vector.memset(v2[:, :, :, 64:128], 1.0)

            # ---------------- input DMAs (priority ~ emission order) -------
            # wq then x(sc0) first: the first q matmul group needs exactly
            # those two, so compute starts early while wk/wv stream behind.
            nc.sync.dma_start(wk_sb[:, :, 0:128], wk[:, :, 0:128])
            xc0 = xpool.tile([128, KC, 512], bf16, tag="xc")
            nc.sync.dma_start(xc0[:, 0:4, :], xp[:, 0, 0:4, :])
            nc.sync.dma_start(wq_sb[:, :, 0:128], wq[:, :, 0:128])
            nc.sync.dma_start(xc0[:, 4:KC, :], xp[:, 0, 4:KC, :])
            nc.sync.dma_start(bias_sb[:], bias[:])

            # PE warmup: ~7 dummy matmuls run while the prologue DMAs land,
            # tripping the HAM activity monitor so the first real chains
            # start at 2.4GHz instead of 1.2GHz.
            with tc.tile_pool(name="pwarm", bufs=1, space="PSUM") as pwarm:
                wps = pwarm.tile([128, 512], fp32)
                for _ in range(12):
                    nc.tensor.matmul(
                        wps[:], warm[:, 0:128], warm[:], start=True, stop=True
                    )
                nc.vector.tensor_copy(warm[:], wps[:])
            nc.sync.dma_start(wk_sb[:, :, 128:GD], wk[:, :, 128:GD])
            nc.sync.dma_start(wq_sb[:, :, 128:GD], wq[:, :, 128:GD])
            nc.sync.dma_start(wv_sb[:], wv[:])

            # ---------------- emission helpers ----------------------------
            def emit_xc(sc):
                xc = xpool.tile([128, KC, 512], bf16, tag="xc")
                nc.sync.dma_start(xc[:, 0:4, :], xp[:, sc, 0:4, :])
                nc.sync.dma_start(xc[:, 4:KC, :], xp[:, sc, 4:KC, :])
                return xc

            def emit_qk(sc, hp, which, xc, paux):
                """One q-or-k projection group for (s-chunk, head pair)."""
                w_sb, dstT, bcol = (
                    (wq_sb, qT, 256) if which == "q" else (wk_sb, kT, 258)
                )
                ps = paux.tile([128, 512], fp32, tag="aux")
                for kc in range(KC):
                    nc.tensor.matmul(
                        ps[:],
                        w_sb[:, kc, ts(hp, 128)],
                        xc[:, kc, :],
                        start=(kc == 0),
                        stop=(kc == KC - 1),
                    )
                nc.vector.tensor_tensor(
                    dstT[:, hp, ts(sc, 512)],
                    ps[:],
                    bias_sb[:, bcol + hp : bcol + hp + 1].to_broadcast(
                        [128, 512]
                    ),
                    Add,
                )

            def qk_thunks(sc, hp, which, xc, paux):
                """A q/k projection as two 4-matmul half-chain thunks (the
                PSUM accumulation spans the two emission points), so filler
                insertion never blocks the attention stream for more than
                ~1us."""
                w_sb, dstT, bcol = (
                    (wq_sb, qT, 256) if which == "q" else (wk_sb, kT, 258)
                )
                box = {}
                def t1():
                    ps = paux.tile([128, 512], fp32, tag="aux")
                    box["ps"] = ps
                    for kc in range(4):
                        nc.tensor.matmul(
                            ps[:],
                            w_sb[:, kc, ts(hp, 128)],
                            xc[:, kc, :],
                            start=(kc == 0),
                            stop=False,
                        )
                def t2():
                    ps = box["ps"]
                    for kc in range(4, KC):
                        nc.tensor.matmul(
                            ps[:],
                            w_sb[:, kc, ts(hp, 128)],
                            xc[:, kc, :],
                            start=False,
                            stop=(kc == KC - 1),
                        )
                    nc.vector.tensor_tensor(
                        dstT[:, hp, ts(sc, 512)],
                        ps[:],
                        bias_sb[:, bcol + hp : bcol + hp + 1].to_broadcast(
                            [128, 512]
                        ),
                        Add,
                    )
                return [t1, t2]

            def emit_v(sc, xc, paux):
                for mm in range(4):
                    kb = sc * 4 + mm
                    ps = paux.tile([128, GD], fp32, tag="aux")
                    for kc in range(KC):
                        nc.tensor.matmul(
                            ps[:],
                            xc[:, kc, ts(mm, 128)],
                            wv_sb[:, kc, :],
                            start=(kc == 0),
                            stop=(kc == KC - 1),
                        )
                    nc.vector.tensor_tensor(
                        v2[:, kb, :, 0:DEPTH],
                        ps[:].rearrange("p (h d) -> p h d", h=GH),
                        bias_sb[:, 0:256].rearrange("p (h d) -> p h d", h=GH),
                        Add,
                    )

            def emit_sexp(hp, sqc, kb, pss):
                """Scores pair -> exp for one k-block (needs only qT/kT)."""
                sps = pss.tile([128, 2, 512], fp32, tag="s")
                for a in range(2):
                    nc.tensor.matmul(
                        sps[:, a, :],
                        kT[a * 64 : (a + 1) * 64, hp, ts(kb, 128)],
                        qT[a * 64 : (a + 1) * 64, hp, ts(sqc, 512)],
                        start=True,
                        stop=True,
                    )
                ex = expp.tile([128, 2, 512], bf16, tag="e")
                nc.scalar.activation(ex[:], sps[:], Exp, scale=0.125)
                return ex

            def emit_attnv(hp, kb, oab, ex):
                for a in range(2):
                    nc.tensor.matmul(
                        oab[a][:],
                        v2[:, kb, 2 * hp + a, :],
                        ex[:, a, :],
                        start=(kb == 0),
                        stop=(kb == 15),
                    )

            def endgroup(hp, sqc, oab):
                """Normalize one finished group (off critical path).

                oab rows 0:64 = unnormalized attention out, rows 64:128 =
                softmax denominator (64 ones-columns in v2).  Cross-quadrant
                DVE copy moves the denominator block to partitions 0:64
                (compute ops cannot read cross-quadrant, plain copies can),
                reciprocal_approx_fast (~51 ULP, plenty for softmax), then
                the normalize-mul writes straight into pair-packed outTs
                (out base 0 for head a=0, base 64 for a=1 -- DVE writes to
                either half-quadrant from base-0 inputs).
                """
                for a in range(2):
                    den = rqp.tile([64, 512], fp32, tag="rq")
                    rcp = rqp.tile([64, 512], fp32, tag="rc")
                    nc.vector.tensor_copy(den[:], oab[a][64:128, :])
                    nc.vector.reciprocal_approx_fast(rcp[:], den[:])
                    nc.vector.tensor_tensor(
                        outTs[a * 64 : (a + 1) * 64, hp, ts(sqc, 512)],
                        oab[a][0:64, :],
                        rcp[:],
                        Mult,
                    )

            def proj_pieces(sqc, paux, split=True):
                """Output projection for one sqc as a list of emission thunks.

                With split=True each (m, nn) piece becomes two thunks (one
                matmul each) so interleaved filler work never blocks the
                attention scores for more than ~one matmul.
                """
                pieces = []
                ot2box = {}
                for mm in range(4):
                    m = sqc * 4 + mm
                    for nn in range(2):
                        box = {}
                        def start(m=m, nn=nn, box=box):
                            ps = paux.tile([128, 512], fp32, tag="aux")
                            box["ps"] = ps
                            nc.tensor.matmul(
                                ps[:],
                                outTs[:, 0, ts(m, 128)],
                                wo_sb[:, 0, ts(nn, 512)],
                                start=True,
                                stop=False,
                            )
                        def finish(m=m, nn=nn, box=box, ot2box=ot2box):
                            ps = box["ps"]
                            nc.tensor.matmul(
                                ps[:],
                                outTs[:, 1, ts(m, 128)],
                                wo_sb[:, 1, ts(nn, 512)],
                                start=False,
                                stop=True,
                            )
                            if nn == 0:
                                ot2 = otp.tile([128, 1024], bf16, tag="ot",
                                               name="ot2")
                                ot2box[m] = ot2
                            ot2 = ot2box[m]
                            nc.vector.tensor_copy(ot2[:, ts(nn, 512)], ps[:])
                            if nn == 1:
                                nc.sync.dma_start(
                                    out[m * 128 : (m + 1) * 128, :], ot2[:]
                                )
                        if split:
                            pieces.append(start)
                            pieces.append(finish)
                        else:
                            def whole(s=start, f=finish):
                                s(); f()
                            pieces.append(whole)
                return pieces

            def proj_tail_pieces(paux):
                """sqc3 projection split for the kernel tail: the hp0-half
                matmuls run as fillers inside group (1,3) (start/stop=True,
                result parked in SBUF so no PSUM slot stays held); the
                hp1 halves + DVE add + paired out DMA run after group
                (1,3)'s epilogue."""
                sqc = 3
                starts, finishes = [], []
                ot2box = {}
                for mm in range(4):
                    m = sqc * 4 + mm
                    for nn in range(2):
                        hbox = {}
                        def start(m=m, nn=nn, hbox=hbox):
                            ps = paux.tile([128, 512], fp32, tag="aux")
                            nc.tensor.matmul(
                                ps[:],
                                outTs[:, 0, ts(m, 128)],
                                wo_sb[:, 0, ts(nn, 512)],
                                start=True,
                                stop=True,
                            )
                            ht = htp.tile([128, 512], fp32, tag="ht")
                            nc.vector.tensor_copy(ht[:], ps[:])
                            hbox["ht"] = ht
                        def finish(m=m, nn=nn, hbox=hbox):
                            ps = paux.tile([128, 512], fp32, tag="aux")
                            nc.tensor.matmul(
                                ps[:],
                                outTs[:, 1, ts(m, 128)],
                                wo_sb[:, 1, ts(nn, 512)],
                                start=True,
                                stop=True,
                            )
                            ot = otp.tile([128, 1024], bf16, tag="ot",
                                          name="ot")
                            nc.vector.tensor_tensor(
                                ot[:, 0:512], ps[:], hbox["ht"][:], Add
                            )
                            # unpaired DMA: in the tail, latency beats
                            # descriptor-count -- fire as soon as the add
                            # lands instead of waiting for the m-block pair
                            nc.sync.dma_start(
                                out[m * 128 : (m + 1) * 128, ts(nn, 512)],
                                ot[:, 0:512],
                            )
                        starts.append(start)
                        finishes.append(finish)
                return starts, finishes

            def emit_group(hp, sqc, pss, poab, fillers=()):
                """16 k-blocks + epilogue; one filler per k-block."""
                oab = [
                    poab.tile([128, 512], fp32, tag="o", name=f"o{a}")
                    for a in range(2)
                ]
                fill = list(fillers)
                fi = 0
                for kb in range(16):
                    ex = emit_sexp(hp, sqc, kb, pss)
                    emit_attnv(hp, kb, oab, ex)
                    if fi < len(fill):
                        fill[fi]()
                        fi += 1
                while fi < len(fill):
                    fill[fi]()
                    fi += 1
                endgroup(hp, sqc, oab)

            # ---------------- main emission --------------------------------
            # group (hp0, sqc0) is interleaved into phase 1: its k-block j
            # only needs kT/v for s-chunk j//4.  Units are emitted right
            # after the chunk's q/k projections (before the v projections)
            # so the first exp fires as early as possible; the attn@v
            # matmuls inside each unit wait on the v evacuations via
            # dataflow.
            with (
                tc.tile_pool(name="pss", bufs=2, space="PSUM") as pss,
                tc.tile_pool(name="poab", bufs=2, space="PSUM") as poab,
                tc.tile_pool(name="paux", bufs=2, space="PSUM") as paux,
            ):
                oab00 = [
                    poab.tile([128, 512], fp32, tag="o", name=f"o{a}")
                    for a in range(2)
                ]
                xcs = [xc0] + [emit_xc(sc) for sc in range(1, 4)]
                nc.sync.dma_start(wo_sb[:], wo[:])
                for sc in range(4):
                    xc = xcs[sc]
                    emit_qk(sc, 0, "k", xc, paux)
                    if sc == 0:
                        emit_qk(0, 0, "q", xc, paux)
                    exs = [
                        emit_sexp(0, 0, kb, pss)
                        for kb in range(4 * sc, 4 * sc + 4)
                    ]
                    if sc == 0:
                        emit_qk(0, 1, "k", xc, paux)
                        emit_qk(0, 1, "q", xc, paux)
                    if sc == 2:
                        emit_qk(1, 0, "q", xcs[1], paux)
                    if sc == 3:
                        emit_qk(1, 1, "q", xcs[1], paux)
                    emit_v(sc, xc, paux)
                    for kb, ex in zip(range(4 * sc, 4 * sc + 4), exs):
                        emit_attnv(0, kb, oab00, ex)
                endgroup(0, 0, oab00)

                # group (hp1, sqc0): k(hp1) for s-chunks 1-3 produced just
                # ahead of the k-blocks that consume them.  Later groups are
                # ACT-bound (group matmuls 13.6us < 17.7us of exp), so the
                # filler load is spread to match: q projections fill the
                # sqc1 groups, proj pieces fill the sqc2/3 groups.
                fill10 = (
                    qk_thunks(1, 1, "k", xcs[1], paux)
                    + qk_thunks(2, 1, "k", xcs[2], paux)
                    + qk_thunks(3, 1, "k", xcs[3], paux)
                )
                emit_group(1, 0, pss, poab, fill10)

                emit_group(
                    0, 1, pss, poab,
                    qk_thunks(2, 0, "q", xcs[2], paux)
                    + qk_thunks(2, 1, "q", xcs[2], paux),
                )
                emit_group(
                    1, 1, pss, poab,
                    qk_thunks(3, 0, "q", xcs[3], paux)
                    + qk_thunks(3, 1, "q", xcs[3], paux),
                )
                pp0 = proj_pieces(0, paux)
                emit_group(0, 2, pss, poab, pp0)
                pp1 = proj_pieces(1, paux)
                emit_group(1, 2, pss, poab, pp1)
                pp2 = proj_pieces(2, paux)
                emit_group(0, 3, pss, poab, pp2)
                pp3s, pp3f = proj_tail_pieces(paux)
                emit_group(1, 3, pss, poab, pp3s)
                for t in pp3f:
                    t()

    nc.compile()
    return nc


def _get_nc():
    if "nc" not in _state:
        _state["nc"] = _build()
    return _state["nc"]


def _prep_core_inputs(inputs, Wq, bq, Wk, bk, Wv, bv, Wo, bo):
    """Host-side shard + pack into the exact SBUF layouts (contiguous DMAs)."""
    in_maps = []
    xps = [
        np.ascontiguousarray(
            # [S, D] -> [p, sc, c, s']: x[sc*512+s', c*128+p]
            np.asarray(inputs[b], np.float32)
            .reshape(4, 512, KC, 128)
            .transpose(3, 0, 2, 1)
            .astype(BF16)
        )
        for b in range(2)
    ]
    for c in range(NCORES):
        b, g = divmod(c, 4)
        cols = slice(g * GD, (g + 1) * GD)
        bias_pack = np.empty((128, 260), np.float32)
        bias_pack[:, 0:256] = np.asarray(bv, np.float32)[None, cols]
        bias_pack[:, 256:258] = np.asarray(bq, np.float32)[cols].reshape(2, 128).T
        bias_pack[:, 258:260] = np.asarray(bk, np.float32)[cols].reshape(2, 128).T
        m = {
            "xp": xps[b],
            # [D, GD] -> [p, c, d]: W[c*128+p, d]
            "wq": np.ascontiguousarray(
                np.asarray(Wq, np.float32)[:, cols].reshape(KC, 128, GD).transpose(1, 0, 2).astype(BF16)
            ),
            "wk": np.ascontiguousarray(
                np.asarray(Wk, np.float32)[:, cols].reshape(KC, 128, GD).transpose(1, 0, 2).astype(BF16)
            ),
            "wv": np.ascontiguousarray(
                np.asarray(Wv, np.float32)[:, cols].reshape(KC, 128, GD).transpose(1, 0, 2).astype(BF16)
            ),
            # [GD, D] -> [p, hp, n]: Wo[hp*128+p, n]
            "wo": np.ascontiguousarray(
                np.asarray(Wo, np.float32)[cols, :].reshape(2, 128, D).transpose(1, 0, 2).astype(BF16)
            ),
            "bias": bias_pack,
        }
        in_maps.append(m)
    return in_maps


def run(inputs, Wq, bq, Wk, bk, Wv, bv, Wo, bo, trace=False):
    from concourse.bass_utils import run_bass_kernel_spmd

    nc = _get_nc()
    in_maps = _prep_core_inputs(inputs, Wq, bq, Wk, bk, Wv, bv, Wo, bo)
    res = run_bass_kernel_spmd(
        nc, in_maps, core_ids=list(range(NCORES)), trace=trace
    )
    out = np.zeros((2, S, D), np.float32)
    for c in range(NCORES):
        out[c // 4] += res.results[c]["out"]
    out += np.asarray(bo, np.float32)
    return out, res


def kernel(inputs, Wq, bq, Wk, bk, Wv, bv, Wo, bo):
    out, _ = run(
        np.asarray(inputs, np.float32),
        np.asarray(Wq, np.float32), np.asarray(bq, np.float32),
        np.asarray(Wk, np.float32), np.asarray(bk, np.float32),
        np.asarray(Wv, np.float32), np.asarray(bv, np.float32),
        np.asarray(Wo, np.float32), np.asarray(bo, np.float32),
    )
    return out
